# revision 60
# baseline (speedup 1.0000x reference)
"""Causal varlen self-attention (packed, equal-length) on 8 trn2 NeuronCores.

Sharding: tensor-parallel over heads — 16 heads / 8 cores = 2 heads per core.
Each core computes qkv + RoPE + RMSNorm + causal attention + sigmoid gating for
its 2 heads over all 4096 tokens, plus its partial output projection
(attn_chunk @ Wo_chunk.T).  The host sums the 8 partial outputs.

Per-core pipeline (feature-major q/k: head_dim on partitions):
  - qkv: q,k produced feature-major [d, t]; v (+ the 2 gate logits appended as
    2 extra columns of the v weight block) produced token-major [t, d].
  - RoPE via a 128x128 signed-permutation matmul + elementwise muls; RMSNorm
    partition-reductions via ones-matmuls on the PE.
  - scores computed TRANSPOSED: scoresT[s, t] = k_fin-slices.T @ q_fin so the
    k-side softmax scale folds into the exp's per-partition scale, and the
    transposed probs are exactly what the PV matmul (lhsT = token-major V)
    wants.  Softmax denominator = ones-matmul over the exp tiles.
  - causal mask: diagonal-chunk matmuls are sliced to the unmasked t-range and
    one [128,128] triangle of -1e30 is added before exp.
  - gate and 1/denominator are per-token (free-dim) scales, applied via a
    partition-broadcast SBUF->SBUF DMA then one elementwise multiply.
"""

import sys

sys.path.insert(0, "/opt/trn_rl_repo")

import numpy as np
import ml_dtypes

import concourse.bass as bass
import concourse.tile as tile
from concourse import bacc, mybir
from concourse.bass_utils import run_bass_kernel_spmd

N_TOK, HID, NH, HD = 4096, 2048, 16, 128
SEQ, NSEQ = 1024, 4
NCORES = 8
EPS = 1e-6
F32, BF16, F32R = mybir.dt.float32, mybir.dt.bfloat16, mybir.dt.float32r
BF = ml_dtypes.bfloat16
AF = mybir.ActivationFunctionType

_PATCHED = False


def _patch_tile_drain():
    """walrus in this env allows only ONE sync-wait on a TPB_CTRL instruction;
    spread the TileContext-exit drain's waits across nop instructions."""
    global _PATCHED
    if _PATCHED:
        return
    _PATCHED = True
    from concourse.tile import TileContext
    from concourse.vector_clock import ScopedClock

    def patched(self, tick_clock, wait_clock):
        nc = self.nc
        probe = nc.sync.nop(nofuse=True, hint="drain_waits_probe")
        wait_clock.add_sem_waits(probe.ins, ScopedClock({None: tick_clock.global_clock}))
        waits = list(probe.ins.sync_info.on_wait or [])
        probe.ins.sync_info.on_wait = waits[:1]
        for w in waits[1:]:
            nop = nc.sync.nop(nofuse=True, hint="drain_waits")
            nop.ins.sync_info = mybir.SyncInfo(on_wait=[w], on_update=[])
        nc.sync.drain()
        nc.all_engine_barrier()
        assert self.sems is not None
        popped = nc._tile_sem_poison_stack.pop()
        assert popped is self._sem_poison
        nc.clear_and_free_semaphores(list(self.sems.allocated().values()))
        nc.all_engine_barrier()

    TileContext._drain_and_barrier = patched


def _r(ap):
    return ap.bitcast(F32R)


def build_nc():
    """One SPMD Bass program; all per-core data arrives via ExternalInputs."""
    nc = bacc.Bacc("TRN2", target_bir_lowering=False, debug=False, num_devices=NCORES)

    xt = nc.dram_tensor("xt", [128, 16, N_TOK], BF16, kind="ExternalInput")
    wqk = nc.dram_tensor("wqk", [128, 4, 16, 128], BF16, kind="ExternalInput")
    wvg = nc.dram_tensor("wvg", [128, 16, 258], BF16, kind="ExternalInput")
    wot = nc.dram_tensor("wot", [128, 2, HID], BF16, kind="ExternalInput")
    cs = nc.dram_tensor("cs", [128, 2, SEQ], BF16, kind="ExternalInput")
    csk = nc.dram_tensor("csk", [128, 2, SEQ], BF16, kind="ExternalInput")
    rtm = nc.dram_tensor("rtm", [128, 128], BF16, kind="ExternalInput")
    tri = nc.dram_tensor("tri", [128, 128], BF16, kind="ExternalInput")
    idn = nc.dram_tensor("idn", [128, 128], BF16, kind="ExternalInput")
    w2c = nc.dram_tensor("w2c", [128, 1], F32, kind="ExternalInput")
    gbc = nc.dram_tensor("gbc", [128, 2], F32, kind="ExternalInput")
    onesr = nc.dram_tensor("onesr", [1, 128], F32R, kind="ExternalInput")
    out = nc.dram_tensor("out", [N_TOK, HID], F32, kind="ExternalOutput")
    gate_scr = nc.dram_tensor("gate_scr", [2, N_TOK], F32)

    with tile.TileContext(nc) as tc:
        with (
            tc.tile_pool(name="consts", bufs=1) as consts,
            tc.tile_pool(name="xtp", bufs=3) as xtp,
            tc.tile_pool(name="qkp", bufs=2) as qkp,
            tc.tile_pool(name="vp", bufs=2) as vp,
            tc.tile_pool(name="ropep", bufs=6) as ropep,
            tc.tile_pool(name="scrp", bufs=3) as scrp,
            tc.tile_pool(name="expp", bufs=3) as expp,
            tc.tile_pool(name="attnp", bufs=2) as attnp,
            tc.tile_pool(name="outp", bufs=3) as outp,
            tc.tile_pool(name="bcp", bufs=2) as bcp,
            tc.tile_pool(name="rowp", bufs=2) as rowp,
            tc.tile_pool(name="gsp", bufs=3) as gsp,
            tc.tile_pool(name="projps", bufs=2, space="PSUM") as projps,
            tc.tile_pool(name="bigps", bufs=5, space="PSUM") as bigps,
            tc.tile_pool(name="vecps", bufs=1, space="PSUM") as vecps,
        ):
            # ---- resident constants (spread across DMA queues: the first
            # qkv matmuls are gated on wc, so parallelize its load)
            # allocate const tiles now; DMA emission order is tuned so the
            # first projection chain's bytes (wqk m=0 + x tile 0) land first
            wqk_t = consts.tile([128, 4, 16, 128], BF16)
            wvg_t = consts.tile([128, 16, 258], BF16)
            wot_t = consts.tile([128, 2, HID], BF16)
            cs_t = consts.tile([128, 2, SEQ], BF16)
            csk_t = consts.tile([128, 2, SEQ], BF16)
            rt_t = consts.tile([128, 128], BF16)
            tri_t = consts.tile([128, 128], BF16)
            idn_t = consts.tile([128, 128], BF16)
            w2_t = consts.tile([128, 1], F32)
            gbn_t = consts.tile([128, 2], F32)
            nc.sync.dma_start(out=gbn_t[:], in_=gbc[:])

            def early_consts():
                # emitted after the first x-tile DMA: wqk m=0 + x tile 0 land
                # first on the serial DMA queue, then the rest of the weights.
                # These MUST be emitted before qkv_ntile(0)'s matmuls — Tile
                # tracks RAW deps in emission order.
                for m in range(1, 4):
                    eng = nc.sync if m % 2 == 0 else nc.scalar
                    eng.dma_start(out=wqk_t[:, m], in_=wqk[:, m])
                nc.scalar.dma_start(out=wvg_t[:], in_=wvg[:])

            def late_consts():
                nc.sync.dma_start(out=rt_t[:], in_=rtm[:])
                nc.sync.dma_start(out=cs_t[:], in_=cs[:])
                nc.sync.dma_start(out=csk_t[:], in_=csk[:])
                nc.sync.dma_start(out=tri_t[:], in_=tri[:])
                nc.sync.dma_start(out=idn_t[:], in_=idn[:])
                nc.sync.dma_start(out=w2_t[:], in_=w2c[:])
                nc.scalar.dma_start(out=wot_t[:], in_=wot[:])
            ones_t = consts.tile([128, 1], F32)
            nc.vector.memset(ones_t[:], 1.0)
            ones_bf = consts.tile([128, 1], BF16)
            nc.vector.memset(ones_bf[:], 1.0)
            # ones/HD in bf16 (2^-7, exact): the q-stats matmul yields mean_d
            ones_q = consts.tile([128, 1], BF16)
            nc.vector.memset(ones_q[:], 1.0 / HD)
            eps_t = consts.tile([128, 1], F32)
            nc.vector.memset(eps_t[:], EPS)
            epsh_t = consts.tile([128, 1], F32)
            nc.vector.memset(epsh_t[:], float(HD * EPS))
            inv128_t = consts.tile([128, 1], F32)
            nc.vector.memset(inv128_t[:], 1.0 / HD)
            ones_row = consts.tile([1, 128], F32R)
            nc.sync.dma_start(out=ones_row[:], in_=onesr[:])

            def qkv_ntile(nt, qk, vt):
                """project 512 tokens: q,k feature-major; v+gate token-major."""
                half = nt % 2
                xtile = load_xtile(nt)
                for m in range(4):  # q_h0, q_h1, k_h0, k_h1
                    ps = projps.tile([128, 512], F32, tag="proj")
                    for kc in range(16):
                        nc.tensor.matmul(
                            ps[:],
                            lhsT=wqk_t[:, m, kc, :],
                            rhs=xtile[:, kc, :],
                            start=(kc == 0),
                            stop=(kc == 15),
                        )
                    nc.vector.tensor_copy(
                        out=qk[:, m, half * 512 : (half + 1) * 512], in_=ps[:]
                    )
                for ti in range(4):  # v + gate logits, token-major, 128 tok each
                    ps = projps.tile([128, 512], F32, tag="proj")
                    for kc in range(16):
                        nc.tensor.matmul(
                            ps[:, 0:258],
                            lhsT=xtile[:, kc, ti * 128 : (ti + 1) * 128],
                            rhs=wvg_t[:, kc, :],
                            start=(kc == 0),
                            stop=(kc == 15),
                        )
                    nc.vector.tensor_copy(out=vt[:, half * 4 + ti, :], in_=ps[:, 0:256])
                    # gate as 1+exp(-(z+b)): shares the ACT Exp table with
                    # attention (no LoadActFuncSet thrash); the reciprocal is
                    # folded into the softmax-denominator reciprocal later
                    gsb = gsp.tile([128, 2], F32, tag="gsb")
                    t0 = nt * 512 + ti * 128
                    for h in range(2):
                        nc.scalar.activation(
                            out=gsb[:, h : h + 1],
                            in_=ps[:, 256 + h : 257 + h],
                            func=AF.Exp,
                            bias=gbn_t[:, h : h + 1],
                            scale=-1.0,
                        )
                    nc.vector.tensor_scalar_add(out=gsb[:], in0=gsb[:], scalar1=ones_t[:])
                    for h in range(2):
                        nc.sync.dma_start(
                            out=gate_scr[h : h + 1, t0 : t0 + 128],
                            in_=gsb[:, h : h + 1],
                        )

            def rope_norm(s, h, is_q, qk):
                """RoPE + RMSNorm scale for one head-tensor of one sequence.
                q: returns fin already scaled by sigma_q (broadcast multiply).
                k: returns (fin * norm_w^2, sigma_k per-partition column)."""
                m = h if is_q else 2 + h
                cst = cs_t if is_q else csk_t
                fin = ropep.tile([128, SEQ], BF16, tag="rope")
                sq = scrp.tile([128, SEQ], BF16, tag="sq")
                # rope is a per-pair rotation: it preserves sum_d q^2, so the
                # RMSNorm stats come from PRE-rope values — a chain parallel to
                # the rotation, not serial after it
                nc.gpsimd.tensor_mul(out=sq[:], in0=qk[:, m, :], in1=qk[:, m, :])
                for j in range(2):
                    js = slice(j * 512, (j + 1) * 512)
                    psr = bigps.tile([128, 512], F32, tag="big")
                    nc.tensor.matmul(
                        psr[:], lhsT=rt_t[:], rhs=qk[:, m, js], start=True, stop=True
                    )
                    nc.gpsimd.tensor_mul(
                        out=fin[:, js], in0=qk[:, m, js], in1=cst[:, 0, js]
                    )
                    tmp = scrp.tile([128, 512], F32, tag="rtmp")
                    nc.vector.tensor_mul(out=tmp[:], in0=psr[:], in1=cst[:, 1, js])
                    nc.vector.tensor_add(out=fin[:, js], in0=fin[:, js], in1=tmp[:])
                if is_q:
                    # sigma_q[t] = rsqrt(mean_d(rope_q^2) + eps), free-dim
                    # scale; processed per 512-half so the first scores tile
                    # unblocks as early as possible
                    row = rowp.tile([1, SEQ], F32, tag="qrow")
                    bc = bcp.tile([128, SEQ], F32, tag="bcq")
                    for j in range(2):
                        js = slice(j * 512, (j + 1) * 512)
                        pss = vecps.tile([1, 512], F32, tag="vec")
                        nc.tensor.matmul(
                            pss[:],
                            lhsT=ones_q[:],
                            rhs=sq[:, js],
                            start=True,
                            stop=True,
                        )
                        nc.scalar.activation(
                            out=row[:, js], in_=pss[:], func=AF.Sqrt,
                            bias=eps_t[0:1, :], scale=1.0,
                        )
                        nc.vector.reciprocal(out=row[:, js], in_=row[:, js])
                        nc.gpsimd.partition_broadcast(bc[:, js], row[:, js])
                        nc.vector.tensor_mul(
                            out=fin[:, js], in0=fin[:, js], in1=bc[:, js]
                        )
                    return fin, None
                else:
                    # sigma_k[s] = rsqrt(sum_d + HD*eps) = rstd_k/sqrt(HD),
                    # per-partition column applied inside the exp
                    col = rowp.tile([128, 8], F32, tag="kcol")
                    psc = projps.tile([128, 8], F32, tag="proj")
                    for sc in range(8):
                        nc.tensor.matmul(
                            psc[:, sc : sc + 1],
                            lhsT=sq[:, sc * 128 : (sc + 1) * 128],
                            rhs=ones_bf[:],
                            start=True,
                            stop=True,
                            skip_group_check=True,
                        )
                    nc.scalar.activation(
                        out=col[:], in_=psc[:], func=AF.Sqrt,
                        bias=epsh_t[:], scale=1.0,
                    )
                    nc.vector.reciprocal(out=col[:], in_=col[:])
                    return fin, col

            def attention(s, h, qk, vt, att, qf, kf, kcol):
                grows = []
                for tt in range(2):  # prefetch gate rows (DRAM latency off the chain)
                    grow = rowp.tile([1, 512], F32, tag="grow", name=f"grow{s}_{h}_{tt}")
                    t0 = s * SEQ + tt * 512
                    nc.sync.dma_start(
                        out=grow[:], in_=gate_scr[h : h + 1, t0 : t0 + 512]
                    )
                    grows.append(grow)
                for tt in range(2):
                    nsc = 4 * (tt + 1)
                    expt = expp.tile([128, 8, 512], BF16, tag="expt")
                    pv = bigps.tile([128, 512], F32, tag="big")
                    den = vecps.tile([1, 512], F32, tag="vec")
                    for sc in range(nsc):
                        r = sc - 4 * tt  # >= 0 on diagonal chunks
                        c0 = 128 * r if r > 0 else 0
                        sps = bigps.tile([128, 512], F32, tag="big")
                        nc.tensor.matmul(
                            sps[:, 0 : 512 - c0],
                            lhsT=kf[:, sc * 128 : (sc + 1) * 128],
                            rhs=qf[:, tt * 512 + c0 : (tt + 1) * 512],
                            start=True,
                            stop=(r < 0),
                            skip_group_check=True,
                        )
                        if r >= 0:  # diagonal chunk: accumulate the -1e30
                            # triangle on the PE itself (I.T @ tri) — keeps the
                            # scores->exp chain off the DVE
                            nc.tensor.matmul(
                                sps[:, 0:128],
                                lhsT=idn_t[:],
                                rhs=tri_t[:],
                                start=False,
                                stop=True,
                                skip_group_check=True,
                            )
                        nc.scalar.activation(
                            out=expt[:, sc, c0:512], in_=sps[:, 0 : 512 - c0],
                            func=AF.Exp, scale=kcol[:, sc : sc + 1],
                        )
                        nc.tensor.matmul(
                            den[:, c0:512],
                            lhsT=ones_bf[:],
                            rhs=expt[:, sc, c0:512],
                            start=(sc == 0),
                            stop=(sc == nsc - 1),
                            skip_group_check=True,
                        )
                        nc.tensor.matmul(
                            pv[:, c0:512],
                            lhsT=vt[:, sc, h * 128 : (h + 1) * 128],
                            rhs=expt[:, sc, c0:512],
                            start=(sc == 0),
                            stop=(sc == nsc - 1),
                            skip_group_check=True,
                        )
                    drec = rowp.tile([1, 512], F32, tag="drec")
                    nc.vector.tensor_mul(out=drec[:], in0=den[:], in1=grows[tt][:])
                    nc.vector.reciprocal(out=drec[:], in_=drec[:])
                    bcg = bcp.tile([128, 512], F32, tag="bcg")
                    nc.gpsimd.partition_broadcast(bcg[:], drec[:])
                    nc.vector.tensor_mul(
                        out=att[:, h, tt * 512 : (tt + 1) * 512], in0=pv[:], in1=bcg[:]
                    )

            def wo_proj(s, att):
                for t8 in range(8):
                    ts_ = slice(t8 * 128, (t8 + 1) * 128)
                    for ot in range(4):
                        os_ = slice(ot * 512, (ot + 1) * 512)
                        ps = bigps.tile([128, 512], F32, tag="big")
                        for h in range(2):
                            nc.tensor.matmul(
                                ps[:],
                                lhsT=att[:, h, ts_],
                                rhs=wot_t[:, h, os_],
                                start=(h == 0),
                                stop=(h == 1),
                            )
                        ob = outp.tile([128, 512], F32, tag="ob")
                        nc.scalar.copy(out=ob[:], in_=ps[:])
                        nc.sync.dma_start(
                            out=out[s * SEQ + t8 * 128 : s * SEQ + (t8 + 1) * 128, os_],
                            in_=ob[:],
                        )

            xtiles = {}

            def load_xtile(nt):
                if nt in xtiles:
                    return xtiles[nt]
                xtile = xtp.tile([128, 16, 512], BF16, tag="xtile", name=f"xt{nt}")
                for q in range(4):
                    nc.sync.dma_start(
                        out=xtile[:, 4 * q : 4 * (q + 1), :],
                        in_=xt[:, 4 * q : 4 * (q + 1), nt * 512 : (nt + 1) * 512],
                    )
                xtiles[nt] = xtile
                return xtile

            for s in range(NSEQ):
                qk = qkp.tile([128, 4, SEQ], BF16, tag="qk", name=f"qk{s}")
                vt = vp.tile([128, 8, 256], BF16, tag="v", name=f"v{s}")
                att = attnp.tile([128, 2, SEQ], BF16, tag="attn")
                if s == 0:
                    # interleave the first weight/x chunks so MM(kc=0) starts
                    # after ~2 small DMAs rather than the full 2.5MB
                    nc.sync.dma_start(out=wqk_t[:, 0, 0:4], in_=wqk[:, 0, 0:4])
                    nc.scalar.dma_start(out=wqk_t[:, 0, 4:16], in_=wqk[:, 0, 4:16])
                    load_xtile(0)
                    early_consts()
                qkv_ntile(2 * s, qk, vt)
                if s == 0:
                    late_consts()
                qkv_ntile(2 * s + 1, qk, vt)
                if s + 1 < NSEQ:
                    # prefetch next seq's x tiles: the serial DMA queue is the
                    # real gate on the next projections starting promptly
                    load_xtile(2 * (s + 1))
                    load_xtile(2 * (s + 1) + 1)
                preps = []
                for h in range(2):
                    qf, _ = rope_norm(s, h, True, qk)
                    kf, kcol = rope_norm(s, h, False, qk)
                    preps.append((qf, kf, kcol))
                for h in range(2):
                    attention(s, h, qk, vt, att, *preps[h])
                wo_proj(s, att)

    if not nc.is_finalized():
        nc.finalize()
    return nc


_NC_CACHE = None


def _get_nc():
    global _NC_CACHE
    if _NC_CACHE is None:
        _NC_CACHE = build_nc()
    return _NC_CACHE


def prep_inputs(x, Wqkv, Wo, gate_w, gate_b, norm_w, cos_cache, sin_cache,
                cu_seqlens, max_seqlen, position_ids):
    x = np.asarray(x, np.float32)
    Wqkv = np.asarray(Wqkv, np.float32)
    Wo = np.asarray(Wo, np.float32)
    gate_w = np.asarray(gate_w, np.float32)
    gate_b = np.asarray(gate_b, np.float32)
    norm_w = np.asarray(norm_w, np.float32)
    cos_cache = np.asarray(cos_cache, np.float32)
    sin_cache = np.asarray(sin_cache, np.float32)
    pid = np.asarray(position_ids).astype(np.int64)
    cu = np.asarray(cu_seqlens).astype(np.int64)
    assert int(max_seqlen) == SEQ and x.shape == (N_TOK, HID)
    assert np.array_equal(cu, np.arange(NSEQ + 1, dtype=np.int64) * SEQ)
    assert np.array_equal(pid, np.tile(np.arange(SEQ, dtype=np.int64), NSEQ))

    xtf = np.ascontiguousarray(x.T).reshape(16, 128, N_TOK).transpose(1, 0, 2)
    xtf = np.ascontiguousarray(xtf).astype(BF)

    C = cos_cache[pid[:SEQ]].T  # [64, 1024]
    S = sin_cache[pid[:SEQ]].T
    csf = np.stack(
        [np.concatenate([C, C], 0), np.concatenate([S, S], 0)], axis=1
    ).astype(BF)
    w2 = (norm_w * norm_w).reshape(128, 1).astype(np.float32)
    cskf = (csf.astype(np.float32) * w2[:, None, :]).astype(BF)

    rt = np.zeros((128, 128), np.float32)
    for j in range(64):
        rt[j, 64 + j] = -1.0  # psR[64+j] = -x1[j]
        rt[64 + j, j] = 1.0  # psR[i] = x2[i]
    rt = rt.astype(BF)

    trif = np.where(
        np.arange(128)[:, None] > np.arange(128)[None, :], np.float32(-1e30), 0.0
    ).astype(BF)
    idnf = np.eye(128, dtype=np.float32).astype(BF)
    w2c = (norm_w * norm_w).reshape(128, 1).astype(np.float32)

    in_maps = []
    for c in range(NCORES):
        hs = [2 * c, 2 * c + 1]
        rows = []
        for t in range(3):  # q, k, v row blocks of Wqkv
            for h in hs:
                rows.extend(range(t * HID + h * HD, t * HID + (h + 1) * HD))
        wsel = np.concatenate([Wqkv[rows], gate_w[hs]], axis=0)  # [770, 2048]
        wall = np.ascontiguousarray(wsel.T).reshape(16, 128, 770).transpose(1, 0, 2)
        wqkf = np.ascontiguousarray(
            wall[:, :, 0:512].reshape(128, 16, 4, 128).transpose(0, 2, 1, 3)
        ).astype(BF)  # [128, 4(m), 16(kc), 128]
        wvgf = np.ascontiguousarray(wall[:, :, 512:770]).astype(BF)
        wo_sl = np.ascontiguousarray(Wo[:, c * 256 : (c + 1) * 256].T)
        wotf = wo_sl.reshape(2, 128, HID).transpose(1, 0, 2)
        wotf = np.ascontiguousarray(wotf).astype(BF)
        gbf = np.broadcast_to(-gate_b[hs][None, :], (128, 2)).astype(np.float32)
        gbf = np.ascontiguousarray(gbf)
        in_maps.append(
            {"xt": xtf, "wqk": wqkf, "wvg": wvgf, "wot": wotf, "cs": csf, "rtm": rt,
             "tri": trif, "idn": idnf, "w2c": w2c, "gbc": gbf, "csk": cskf,
             "onesr": np.ones((1, 128), np.float32)}
        )
    return in_maps


def run(inputs, trace=False):
    in_maps = prep_inputs(**inputs)
    nc = _get_nc()
    res = run_bass_kernel_spmd(nc, in_maps, core_ids=list(range(NCORES)), trace=trace)
    total = np.zeros((N_TOK, HID), np.float32)
    for c in range(NCORES):
        total += res.results[c]["out"].astype(np.float32)
    return total, res


def kernel(**inputs) -> np.ndarray:
    out, _ = run(inputs)
    return out


# revision 65
# speedup vs baseline: 1.0075x; 1.0075x over previous
"""Causal varlen self-attention (packed, equal-length) on 8 trn2 NeuronCores.

Sharding: tensor-parallel over heads — 16 heads / 8 cores = 2 heads per core.
Each core computes qkv + RoPE + RMSNorm + causal attention + sigmoid gating for
its 2 heads over all 4096 tokens, plus its partial output projection
(attn_chunk @ Wo_chunk.T).  The host sums the 8 partial outputs.

Per-core pipeline (feature-major q/k: head_dim on partitions):
  - qkv: q,k produced feature-major [d, t]; v (+ the 2 gate logits appended as
    2 extra columns of the v weight block) produced token-major [t, d].
  - RoPE via a 128x128 signed-permutation matmul + elementwise muls; RMSNorm
    partition-reductions via ones-matmuls on the PE.
  - scores computed TRANSPOSED: scoresT[s, t] = k_fin-slices.T @ q_fin so the
    k-side softmax scale folds into the exp's per-partition scale, and the
    transposed probs are exactly what the PV matmul (lhsT = token-major V)
    wants.  Softmax denominator = ones-matmul over the exp tiles.
  - causal mask: diagonal-chunk matmuls are sliced to the unmasked t-range and
    one [128,128] triangle of -1e30 is added before exp.
  - gate and 1/denominator are per-token (free-dim) scales, applied via a
    partition-broadcast SBUF->SBUF DMA then one elementwise multiply.
"""

import sys

sys.path.insert(0, "/opt/trn_rl_repo")

import numpy as np
import ml_dtypes

import concourse.bass as bass
import concourse.tile as tile
from concourse import bacc, mybir
from concourse.bass_utils import run_bass_kernel_spmd

N_TOK, HID, NH, HD = 4096, 2048, 16, 128
SEQ, NSEQ = 1024, 4
NCORES = 8
EPS = 1e-6
F32, BF16, F32R = mybir.dt.float32, mybir.dt.bfloat16, mybir.dt.float32r
BF = ml_dtypes.bfloat16
AF = mybir.ActivationFunctionType

_PATCHED = False


def _patch_tile_drain():
    """walrus in this env allows only ONE sync-wait on a TPB_CTRL instruction;
    spread the TileContext-exit drain's waits across nop instructions."""
    global _PATCHED
    if _PATCHED:
        return
    _PATCHED = True
    from concourse.tile import TileContext
    from concourse.vector_clock import ScopedClock

    def patched(self, tick_clock, wait_clock):
        nc = self.nc
        probe = nc.sync.nop(nofuse=True, hint="drain_waits_probe")
        wait_clock.add_sem_waits(probe.ins, ScopedClock({None: tick_clock.global_clock}))
        waits = list(probe.ins.sync_info.on_wait or [])
        probe.ins.sync_info.on_wait = waits[:1]
        for w in waits[1:]:
            nop = nc.sync.nop(nofuse=True, hint="drain_waits")
            nop.ins.sync_info = mybir.SyncInfo(on_wait=[w], on_update=[])
        nc.sync.drain()
        nc.all_engine_barrier()
        assert self.sems is not None
        popped = nc._tile_sem_poison_stack.pop()
        assert popped is self._sem_poison
        nc.clear_and_free_semaphores(list(self.sems.allocated().values()))
        nc.all_engine_barrier()

    TileContext._drain_and_barrier = patched


def _r(ap):
    return ap.bitcast(F32R)


def build_nc():
    """One SPMD Bass program; all per-core data arrives via ExternalInputs."""
    nc = bacc.Bacc("TRN2", target_bir_lowering=False, debug=False, num_devices=NCORES)

    xt = nc.dram_tensor("xt", [128, 16, N_TOK], BF16, kind="ExternalInput")
    wqk = nc.dram_tensor("wqk", [128, 4, 16, 128], BF16, kind="ExternalInput")
    wvg = nc.dram_tensor("wvg", [128, 16, 258], BF16, kind="ExternalInput")
    wot = nc.dram_tensor("wot", [128, 2, HID], BF16, kind="ExternalInput")
    cs = nc.dram_tensor("cs", [128, 2, SEQ], BF16, kind="ExternalInput")
    csk = nc.dram_tensor("csk", [128, 2, SEQ], BF16, kind="ExternalInput")
    rtm = nc.dram_tensor("rtm", [128, 128], BF16, kind="ExternalInput")
    tri = nc.dram_tensor("tri", [128, 128], BF16, kind="ExternalInput")
    idn = nc.dram_tensor("idn", [128, 128], BF16, kind="ExternalInput")
    w2c = nc.dram_tensor("w2c", [128, 1], F32, kind="ExternalInput")
    gbc = nc.dram_tensor("gbc", [128, 2], F32, kind="ExternalInput")
    onesr = nc.dram_tensor("onesr", [1, 128], F32R, kind="ExternalInput")
    out = nc.dram_tensor("out", [N_TOK, HID], F32, kind="ExternalOutput")
    gate_scr = nc.dram_tensor("gate_scr", [2, N_TOK], F32)

    with tile.TileContext(nc) as tc:
        with (
            tc.tile_pool(name="consts", bufs=1) as consts,
            tc.tile_pool(name="xtp", bufs=3) as xtp,
            tc.tile_pool(name="qkp", bufs=2) as qkp,
            tc.tile_pool(name="vp", bufs=2) as vp,
            tc.tile_pool(name="ropep", bufs=6) as ropep,
            tc.tile_pool(name="scrp", bufs=3) as scrp,
            tc.tile_pool(name="expp", bufs=3) as expp,
            tc.tile_pool(name="attnp", bufs=2) as attnp,
            tc.tile_pool(name="outp", bufs=3) as outp,
            tc.tile_pool(name="bcp", bufs=2) as bcp,
            tc.tile_pool(name="rowp", bufs=2) as rowp,
            tc.tile_pool(name="gsp", bufs=3) as gsp,
            tc.tile_pool(name="projps", bufs=2, space="PSUM") as projps,
            tc.tile_pool(name="bigps", bufs=5, space="PSUM") as bigps,
            tc.tile_pool(name="vecps", bufs=1, space="PSUM") as vecps,
        ):
            # ---- resident constants (spread across DMA queues: the first
            # qkv matmuls are gated on wc, so parallelize its load)
            # allocate const tiles now; DMA emission order is tuned so the
            # first projection chain's bytes (wqk m=0 + x tile 0) land first
            wqk_t = consts.tile([128, 4, 16, 128], BF16)
            wvg_t = consts.tile([128, 16, 258], BF16)
            wot_t = consts.tile([128, 2, HID], BF16)
            cs_t = consts.tile([128, 2, SEQ], BF16)
            csk_t = consts.tile([128, 2, SEQ], BF16)
            rt_t = consts.tile([128, 128], BF16)
            tri_t = consts.tile([128, 128], BF16)
            idn_t = consts.tile([128, 128], BF16)
            w2_t = consts.tile([128, 1], F32)
            gbn_t = consts.tile([128, 2], F32)
            nc.sync.dma_start(out=gbn_t[:], in_=gbc[:])

            def early_consts():
                # emitted after the first x-tile DMA: wqk m=0 + x tile 0 land
                # first on the serial DMA queue, then the rest of the weights.
                # These MUST be emitted before qkv_ntile(0)'s matmuls — Tile
                # tracks RAW deps in emission order.
                for m in range(1, 4):
                    eng = nc.sync if m % 2 == 0 else nc.scalar
                    eng.dma_start(out=wqk_t[:, m], in_=wqk[:, m])
                nc.scalar.dma_start(out=wvg_t[:], in_=wvg[:])

            def late_consts():
                nc.scalar.dma_start(out=rt_t[:], in_=rtm[:])
                nc.scalar.dma_start(out=cs_t[:], in_=cs[:])
                nc.scalar.dma_start(out=csk_t[:], in_=csk[:])
                nc.scalar.dma_start(out=tri_t[:], in_=tri[:])
                nc.scalar.dma_start(out=idn_t[:], in_=idn[:])
                nc.scalar.dma_start(out=w2_t[:], in_=w2c[:])
                nc.scalar.dma_start(out=wot_t[:], in_=wot[:])
            ones_t = consts.tile([128, 1], F32)
            nc.vector.memset(ones_t[:], 1.0)
            ones_bf = consts.tile([128, 1], BF16)
            nc.vector.memset(ones_bf[:], 1.0)
            # ones/HD in bf16 (2^-7, exact): the q-stats matmul yields mean_d
            ones_q = consts.tile([128, 1], BF16)
            nc.vector.memset(ones_q[:], 1.0 / HD)
            eps_t = consts.tile([128, 1], F32)
            nc.vector.memset(eps_t[:], EPS)
            epsh_t = consts.tile([128, 1], F32)
            nc.vector.memset(epsh_t[:], float(HD * EPS))
            inv128_t = consts.tile([128, 1], F32)
            nc.vector.memset(inv128_t[:], 1.0 / HD)
            ones_row = consts.tile([1, 128], F32R)
            nc.sync.dma_start(out=ones_row[:], in_=onesr[:])

            def qkv_ntile(nt, qk, vt):
                """project 512 tokens: q,k feature-major; v+gate token-major."""
                half = nt % 2
                xtile = load_xtile(nt)
                for m in range(4):  # q_h0, q_h1, k_h0, k_h1
                    ps = projps.tile([128, 512], F32, tag="proj")
                    for kc in range(16):
                        nc.tensor.matmul(
                            ps[:],
                            lhsT=wqk_t[:, m, kc, :],
                            rhs=xtile[:, kc, :],
                            start=(kc == 0),
                            stop=(kc == 15),
                        )
                    nc.vector.tensor_copy(
                        out=qk[:, m, half * 512 : (half + 1) * 512], in_=ps[:]
                    )
                for ti in range(4):  # v + gate logits, token-major, 128 tok each
                    ps = projps.tile([128, 512], F32, tag="proj")
                    for kc in range(16):
                        nc.tensor.matmul(
                            ps[:, 0:258],
                            lhsT=xtile[:, kc, ti * 128 : (ti + 1) * 128],
                            rhs=wvg_t[:, kc, :],
                            start=(kc == 0),
                            stop=(kc == 15),
                        )
                    nc.vector.tensor_copy(out=vt[:, half * 4 + ti, :], in_=ps[:, 0:256])
                    # gate as 1+exp(-(z+b)): shares the ACT Exp table with
                    # attention (no LoadActFuncSet thrash); the reciprocal is
                    # folded into the softmax-denominator reciprocal later
                    gsb = gsp.tile([128, 2], F32, tag="gsb")
                    t0 = nt * 512 + ti * 128
                    for h in range(2):
                        nc.scalar.activation(
                            out=gsb[:, h : h + 1],
                            in_=ps[:, 256 + h : 257 + h],
                            func=AF.Exp,
                            bias=gbn_t[:, h : h + 1],
                            scale=-1.0,
                        )
                    nc.vector.tensor_scalar_add(out=gsb[:], in0=gsb[:], scalar1=ones_t[:])
                    for h in range(2):
                        nc.sync.dma_start(
                            out=gate_scr[h : h + 1, t0 : t0 + 128],
                            in_=gsb[:, h : h + 1],
                        )

            def rope_norm(s, h, is_q, qk):
                """RoPE + RMSNorm scale for one head-tensor of one sequence.
                q: returns fin already scaled by sigma_q (broadcast multiply).
                k: returns (fin * norm_w^2, sigma_k per-partition column)."""
                m = h if is_q else 2 + h
                cst = cs_t if is_q else csk_t
                fin = ropep.tile([128, SEQ], BF16, tag="rope")
                sq = scrp.tile([128, SEQ], BF16, tag="sq")
                # rope is a per-pair rotation: it preserves sum_d q^2, so the
                # RMSNorm stats come from PRE-rope values — a chain parallel to
                # the rotation, not serial after it
                nc.gpsimd.tensor_mul(out=sq[:], in0=qk[:, m, :], in1=qk[:, m, :])
                for j in range(2):
                    js = slice(j * 512, (j + 1) * 512)
                    psr = bigps.tile([128, 512], F32, tag="big")
                    nc.tensor.matmul(
                        psr[:], lhsT=rt_t[:], rhs=qk[:, m, js], start=True, stop=True
                    )
                    nc.gpsimd.tensor_mul(
                        out=fin[:, js], in0=qk[:, m, js], in1=cst[:, 0, js]
                    )
                    tmp = scrp.tile([128, 512], F32, tag="rtmp")
                    nc.vector.tensor_mul(out=tmp[:], in0=psr[:], in1=cst[:, 1, js])
                    nc.vector.tensor_add(out=fin[:, js], in0=fin[:, js], in1=tmp[:])
                if is_q:
                    # sigma_q[t] = rsqrt(mean_d(rope_q^2) + eps), free-dim
                    # scale; processed per 512-half so the first scores tile
                    # unblocks as early as possible
                    row = rowp.tile([1, SEQ], F32, tag="qrow")
                    bc = bcp.tile([128, SEQ], F32, tag="bcq")
                    for j in range(2):
                        js = slice(j * 512, (j + 1) * 512)
                        pss = vecps.tile([1, 512], F32, tag="vec")
                        nc.tensor.matmul(
                            pss[:],
                            lhsT=ones_q[:],
                            rhs=sq[:, js],
                            start=True,
                            stop=True,
                        )
                        nc.scalar.activation(
                            out=row[:, js], in_=pss[:], func=AF.Sqrt,
                            bias=eps_t[0:1, :], scale=1.0,
                        )
                        nc.vector.reciprocal(out=row[:, js], in_=row[:, js])
                        nc.gpsimd.partition_broadcast(bc[:, js], row[:, js])
                        nc.vector.tensor_mul(
                            out=fin[:, js], in0=fin[:, js], in1=bc[:, js]
                        )
                    return fin, None
                else:
                    # sigma_k[s] = rsqrt(sum_d + HD*eps) = rstd_k/sqrt(HD),
                    # per-partition column applied inside the exp
                    col = rowp.tile([128, 8], F32, tag="kcol")
                    psc = projps.tile([128, 8], F32, tag="proj")
                    for sc in range(8):
                        nc.tensor.matmul(
                            psc[:, sc : sc + 1],
                            lhsT=sq[:, sc * 128 : (sc + 1) * 128],
                            rhs=ones_bf[:],
                            start=True,
                            stop=True,
                            skip_group_check=True,
                        )
                    nc.scalar.activation(
                        out=col[:], in_=psc[:], func=AF.Sqrt,
                        bias=epsh_t[:], scale=1.0,
                    )
                    nc.vector.reciprocal(out=col[:], in_=col[:])
                    return fin, col

            def attention(s, h, qk, vt, att, qf, kf, kcol):
                grows = []
                for tt in range(2):  # prefetch gate rows (DRAM latency off the chain)
                    grow = rowp.tile([1, 512], F32, tag="grow", name=f"grow{s}_{h}_{tt}")
                    t0 = s * SEQ + tt * 512
                    nc.sync.dma_start(
                        out=grow[:], in_=gate_scr[h : h + 1, t0 : t0 + 512]
                    )
                    grows.append(grow)
                for tt in range(2):
                    nsc = 4 * (tt + 1)
                    expt = expp.tile([128, 8, 512], BF16, tag="expt")
                    pv = bigps.tile([128, 512], F32, tag="big")
                    den = vecps.tile([1, 512], F32, tag="vec")
                    for sc in range(nsc):
                        r = sc - 4 * tt  # >= 0 on diagonal chunks
                        c0 = 128 * r if r > 0 else 0
                        sps = bigps.tile([128, 512], F32, tag="big")
                        nc.tensor.matmul(
                            sps[:, 0 : 512 - c0],
                            lhsT=kf[:, sc * 128 : (sc + 1) * 128],
                            rhs=qf[:, tt * 512 + c0 : (tt + 1) * 512],
                            start=True,
                            stop=(r < 0),
                            skip_group_check=True,
                        )
                        if r >= 0:  # diagonal chunk: accumulate the -1e30
                            # triangle on the PE itself (I.T @ tri) — keeps the
                            # scores->exp chain off the DVE
                            nc.tensor.matmul(
                                sps[:, 0:128],
                                lhsT=idn_t[:],
                                rhs=tri_t[:],
                                start=False,
                                stop=True,
                                skip_group_check=True,
                            )
                        nc.scalar.activation(
                            out=expt[:, sc, c0:512], in_=sps[:, 0 : 512 - c0],
                            func=AF.Exp, scale=kcol[:, sc : sc + 1],
                        )
                        nc.tensor.matmul(
                            den[:, c0:512],
                            lhsT=ones_bf[:],
                            rhs=expt[:, sc, c0:512],
                            start=(sc == 0),
                            stop=(sc == nsc - 1),
                            skip_group_check=True,
                        )
                        nc.tensor.matmul(
                            pv[:, c0:512],
                            lhsT=vt[:, sc, h * 128 : (h + 1) * 128],
                            rhs=expt[:, sc, c0:512],
                            start=(sc == 0),
                            stop=(sc == nsc - 1),
                            skip_group_check=True,
                        )
                    drec = rowp.tile([1, 512], F32, tag="drec")
                    nc.vector.tensor_mul(out=drec[:], in0=den[:], in1=grows[tt][:])
                    nc.vector.reciprocal(out=drec[:], in_=drec[:])
                    bcg = bcp.tile([128, 512], F32, tag="bcg")
                    nc.gpsimd.partition_broadcast(bcg[:], drec[:])
                    nc.vector.tensor_mul(
                        out=att[:, h, tt * 512 : (tt + 1) * 512], in0=pv[:], in1=bcg[:]
                    )

            def wo_proj(s, att):
                for t8 in range(8):
                    ts_ = slice(t8 * 128, (t8 + 1) * 128)
                    for ot in range(4):
                        os_ = slice(ot * 512, (ot + 1) * 512)
                        ps = bigps.tile([128, 512], F32, tag="big")
                        for h in range(2):
                            nc.tensor.matmul(
                                ps[:],
                                lhsT=att[:, h, ts_],
                                rhs=wot_t[:, h, os_],
                                start=(h == 0),
                                stop=(h == 1),
                            )
                        ob = outp.tile([128, 512], F32, tag="ob")
                        nc.scalar.copy(out=ob[:], in_=ps[:])
                        nc.sync.dma_start(
                            out=out[s * SEQ + t8 * 128 : s * SEQ + (t8 + 1) * 128, os_],
                            in_=ob[:],
                        )

            xtiles = {}

            def load_xtile(nt):
                if nt in xtiles:
                    return xtiles[nt]
                xtile = xtp.tile([128, 16, 512], BF16, tag="xtile", name=f"xt{nt}")
                for q in range(4):
                    nc.sync.dma_start(
                        out=xtile[:, 4 * q : 4 * (q + 1), :],
                        in_=xt[:, 4 * q : 4 * (q + 1), nt * 512 : (nt + 1) * 512],
                    )
                xtiles[nt] = xtile
                return xtile

            for s in range(NSEQ):
                qk = qkp.tile([128, 4, SEQ], BF16, tag="qk", name=f"qk{s}")
                vt = vp.tile([128, 8, 256], BF16, tag="v", name=f"v{s}")
                att = attnp.tile([128, 2, SEQ], BF16, tag="attn")
                if s == 0:
                    # interleave the first weight/x chunks so MM(kc=0) starts
                    # after ~2 small DMAs rather than the full 2.5MB
                    nc.sync.dma_start(out=wqk_t[:, 0, 0:4], in_=wqk[:, 0, 0:4])
                    nc.scalar.dma_start(out=wqk_t[:, 0, 4:16], in_=wqk[:, 0, 4:16])
                    load_xtile(0)
                    early_consts()
                qkv_ntile(2 * s, qk, vt)
                if s == 0:
                    late_consts()
                qkv_ntile(2 * s + 1, qk, vt)
                if s + 1 < NSEQ:
                    # prefetch next seq's x tiles: the serial DMA queue is the
                    # real gate on the next projections starting promptly
                    load_xtile(2 * (s + 1))
                    load_xtile(2 * (s + 1) + 1)
                preps = []
                for h in range(2):
                    qf, _ = rope_norm(s, h, True, qk)
                    kf, kcol = rope_norm(s, h, False, qk)
                    preps.append((qf, kf, kcol))
                for h in range(2):
                    attention(s, h, qk, vt, att, *preps[h])
                wo_proj(s, att)

    if not nc.is_finalized():
        nc.finalize()
    return nc


_NC_CACHE = None


def _get_nc():
    global _NC_CACHE
    if _NC_CACHE is None:
        _NC_CACHE = build_nc()
    return _NC_CACHE


def prep_inputs(x, Wqkv, Wo, gate_w, gate_b, norm_w, cos_cache, sin_cache,
                cu_seqlens, max_seqlen, position_ids):
    x = np.asarray(x, np.float32)
    Wqkv = np.asarray(Wqkv, np.float32)
    Wo = np.asarray(Wo, np.float32)
    gate_w = np.asarray(gate_w, np.float32)
    gate_b = np.asarray(gate_b, np.float32)
    norm_w = np.asarray(norm_w, np.float32)
    cos_cache = np.asarray(cos_cache, np.float32)
    sin_cache = np.asarray(sin_cache, np.float32)
    pid = np.asarray(position_ids).astype(np.int64)
    cu = np.asarray(cu_seqlens).astype(np.int64)
    assert int(max_seqlen) == SEQ and x.shape == (N_TOK, HID)
    assert np.array_equal(cu, np.arange(NSEQ + 1, dtype=np.int64) * SEQ)
    assert np.array_equal(pid, np.tile(np.arange(SEQ, dtype=np.int64), NSEQ))

    xtf = np.ascontiguousarray(x.T).reshape(16, 128, N_TOK).transpose(1, 0, 2)
    xtf = np.ascontiguousarray(xtf).astype(BF)

    C = cos_cache[pid[:SEQ]].T  # [64, 1024]
    S = sin_cache[pid[:SEQ]].T
    csf = np.stack(
        [np.concatenate([C, C], 0), np.concatenate([S, S], 0)], axis=1
    ).astype(BF)
    w2 = (norm_w * norm_w).reshape(128, 1).astype(np.float32)
    cskf = (csf.astype(np.float32) * w2[:, None, :]).astype(BF)

    rt = np.zeros((128, 128), np.float32)
    for j in range(64):
        rt[j, 64 + j] = -1.0  # psR[64+j] = -x1[j]
        rt[64 + j, j] = 1.0  # psR[i] = x2[i]
    rt = rt.astype(BF)

    trif = np.where(
        np.arange(128)[:, None] > np.arange(128)[None, :], np.float32(-1e30), 0.0
    ).astype(BF)
    idnf = np.eye(128, dtype=np.float32).astype(BF)
    w2c = (norm_w * norm_w).reshape(128, 1).astype(np.float32)

    in_maps = []
    for c in range(NCORES):
        hs = [2 * c, 2 * c + 1]
        rows = []
        for t in range(3):  # q, k, v row blocks of Wqkv
            for h in hs:
                rows.extend(range(t * HID + h * HD, t * HID + (h + 1) * HD))
        wsel = np.concatenate([Wqkv[rows], gate_w[hs]], axis=0)  # [770, 2048]
        wall = np.ascontiguousarray(wsel.T).reshape(16, 128, 770).transpose(1, 0, 2)
        wqkf = np.ascontiguousarray(
            wall[:, :, 0:512].reshape(128, 16, 4, 128).transpose(0, 2, 1, 3)
        ).astype(BF)  # [128, 4(m), 16(kc), 128]
        wvgf = np.ascontiguousarray(wall[:, :, 512:770]).astype(BF)
        wo_sl = np.ascontiguousarray(Wo[:, c * 256 : (c + 1) * 256].T)
        wotf = wo_sl.reshape(2, 128, HID).transpose(1, 0, 2)
        wotf = np.ascontiguousarray(wotf).astype(BF)
        gbf = np.broadcast_to(-gate_b[hs][None, :], (128, 2)).astype(np.float32)
        gbf = np.ascontiguousarray(gbf)
        in_maps.append(
            {"xt": xtf, "wqk": wqkf, "wvg": wvgf, "wot": wotf, "cs": csf, "rtm": rt,
             "tri": trif, "idn": idnf, "w2c": w2c, "gbc": gbf, "csk": cskf,
             "onesr": np.ones((1, 128), np.float32)}
        )
    return in_maps


def run(inputs, trace=False):
    in_maps = prep_inputs(**inputs)
    nc = _get_nc()
    res = run_bass_kernel_spmd(nc, in_maps, core_ids=list(range(NCORES)), trace=trace)
    total = np.zeros((N_TOK, HID), np.float32)
    for c in range(NCORES):
        total += res.results[c]["out"].astype(np.float32)
    return total, res


def kernel(**inputs) -> np.ndarray:
    out, _ = run(inputs)
    return out


# revision 68
# speedup vs baseline: 1.0097x; 1.0022x over previous
"""Causal varlen self-attention (packed, equal-length) on 8 trn2 NeuronCores.

Sharding: tensor-parallel over heads — 16 heads / 8 cores = 2 heads per core.
Each core computes qkv + RoPE + RMSNorm + causal attention + sigmoid gating for
its 2 heads over all 4096 tokens, plus its partial output projection
(attn_chunk @ Wo_chunk.T).  The host sums the 8 partial outputs.

Per-core pipeline (feature-major q/k: head_dim on partitions):
  - qkv: q,k produced feature-major [d, t]; v (+ the 2 gate logits appended as
    2 extra columns of the v weight block) produced token-major [t, d].
  - RoPE via a 128x128 signed-permutation matmul + elementwise muls; RMSNorm
    partition-reductions via ones-matmuls on the PE.
  - scores computed TRANSPOSED: scoresT[s, t] = k_fin-slices.T @ q_fin so the
    k-side softmax scale folds into the exp's per-partition scale, and the
    transposed probs are exactly what the PV matmul (lhsT = token-major V)
    wants.  Softmax denominator = ones-matmul over the exp tiles.
  - causal mask: diagonal-chunk matmuls are sliced to the unmasked t-range and
    one [128,128] triangle of -1e30 is added before exp.
  - gate and 1/denominator are per-token (free-dim) scales, applied via a
    partition-broadcast SBUF->SBUF DMA then one elementwise multiply.
"""

import sys

sys.path.insert(0, "/opt/trn_rl_repo")

import numpy as np
import ml_dtypes

import concourse.bass as bass
import concourse.tile as tile
from concourse import bacc, mybir
from concourse.bass_utils import run_bass_kernel_spmd

N_TOK, HID, NH, HD = 4096, 2048, 16, 128
SEQ, NSEQ = 1024, 4
NCORES = 8
EPS = 1e-6
F32, BF16, F32R = mybir.dt.float32, mybir.dt.bfloat16, mybir.dt.float32r
BF = ml_dtypes.bfloat16
AF = mybir.ActivationFunctionType

_PATCHED = False


def _patch_tile_drain():
    """walrus in this env allows only ONE sync-wait on a TPB_CTRL instruction;
    spread the TileContext-exit drain's waits across nop instructions."""
    global _PATCHED
    if _PATCHED:
        return
    _PATCHED = True
    from concourse.tile import TileContext
    from concourse.vector_clock import ScopedClock

    def patched(self, tick_clock, wait_clock):
        nc = self.nc
        probe = nc.sync.nop(nofuse=True, hint="drain_waits_probe")
        wait_clock.add_sem_waits(probe.ins, ScopedClock({None: tick_clock.global_clock}))
        raw = list(probe.ins.sync_info.on_wait or [])
        best = {}
        for w in raw:  # keep one wait per semaphore (the largest threshold)
            k = (w.id, w.wait_mode)
            if k not in best or (w.wait_value or 0) > (best[k].wait_value or 0):
                best[k] = w
        waits = list(best.values())
        probe.ins.sync_info.on_wait = waits[:1]
        for w in waits[1:]:
            nop = nc.sync.nop(nofuse=True, hint="drain_waits")
            nop.ins.sync_info = mybir.SyncInfo(on_wait=[w], on_update=[])
        nc.sync.drain()
        nc.all_engine_barrier()
        assert self.sems is not None
        popped = nc._tile_sem_poison_stack.pop()
        assert popped is self._sem_poison
        nc.clear_and_free_semaphores(list(self.sems.allocated().values()))
        nc.all_engine_barrier()

    TileContext._drain_and_barrier = patched


def _r(ap):
    return ap.bitcast(F32R)


def build_nc():
    """One SPMD Bass program; all per-core data arrives via ExternalInputs."""
    nc = bacc.Bacc("TRN2", target_bir_lowering=False, debug=False, num_devices=NCORES)

    xt = nc.dram_tensor("xt", [128, 16, N_TOK], BF16, kind="ExternalInput")
    wqk = nc.dram_tensor("wqk", [128, 4, 16, 128], BF16, kind="ExternalInput")
    wvg = nc.dram_tensor("wvg", [128, 16, 258], BF16, kind="ExternalInput")
    wot = nc.dram_tensor("wot", [128, 2, HID], BF16, kind="ExternalInput")
    cs = nc.dram_tensor("cs", [128, 2, SEQ], BF16, kind="ExternalInput")
    csk = nc.dram_tensor("csk", [128, 2, SEQ], BF16, kind="ExternalInput")
    rtm = nc.dram_tensor("rtm", [128, 128], BF16, kind="ExternalInput")
    tri = nc.dram_tensor("tri", [128, 128], BF16, kind="ExternalInput")
    idn = nc.dram_tensor("idn", [128, 128], BF16, kind="ExternalInput")
    w2c = nc.dram_tensor("w2c", [128, 1], F32, kind="ExternalInput")
    gbc = nc.dram_tensor("gbc", [128, 2], F32, kind="ExternalInput")
    onesr = nc.dram_tensor("onesr", [1, 128], F32R, kind="ExternalInput")
    out = nc.dram_tensor("out", [N_TOK, HID], F32, kind="ExternalOutput")
    gate_scr = nc.dram_tensor("gate_scr", [2, N_TOK], F32)

    with tile.TileContext(nc) as tc:
        with (
            tc.tile_pool(name="consts", bufs=1) as consts,
            tc.tile_pool(name="xtp", bufs=3) as xtp,
            tc.tile_pool(name="qkp", bufs=2) as qkp,
            tc.tile_pool(name="vp", bufs=2) as vp,
            tc.tile_pool(name="ropep", bufs=6) as ropep,
            tc.tile_pool(name="scrp", bufs=3) as scrp,
            tc.tile_pool(name="expp", bufs=3) as expp,
            tc.tile_pool(name="attnp", bufs=2) as attnp,
            tc.tile_pool(name="outp", bufs=3) as outp,
            tc.tile_pool(name="bcp", bufs=2) as bcp,
            tc.tile_pool(name="rowp", bufs=2) as rowp,
            tc.tile_pool(name="gsp", bufs=3) as gsp,
            tc.tile_pool(name="projps", bufs=2, space="PSUM") as projps,
            tc.tile_pool(name="bigps", bufs=5, space="PSUM") as bigps,
            tc.tile_pool(name="vecps", bufs=1, space="PSUM") as vecps,
        ):
            # ---- resident constants (spread across DMA queues: the first
            # qkv matmuls are gated on wc, so parallelize its load)
            # allocate const tiles now; DMA emission order is tuned so the
            # first projection chain's bytes (wqk m=0 + x tile 0) land first
            wqk_t = consts.tile([128, 4, 16, 128], BF16)
            wvg_t = consts.tile([128, 16, 258], BF16)
            wot_t = consts.tile([128, 2, HID], BF16)
            cs_t = consts.tile([128, 2, SEQ], BF16)
            csk_t = consts.tile([128, 2, SEQ], BF16)
            rt_t = consts.tile([128, 128], BF16)
            tri_t = consts.tile([128, 128], BF16)
            idn_t = consts.tile([128, 128], BF16)
            w2_t = consts.tile([128, 1], F32)
            gbn_t = consts.tile([128, 2], F32)
            nc.sync.dma_start(out=gbn_t[:], in_=gbc[:])

            def early_consts():
                # emitted after the first x-tile DMA: wqk m=0 + x tile 0 land
                # first on the serial DMA queue, then the rest of the weights.
                # These MUST be emitted before qkv_ntile(0)'s matmuls — Tile
                # tracks RAW deps in emission order.
                for m in range(1, 4):
                    eng = nc.sync if m % 2 == 0 else nc.scalar
                    eng.dma_start(out=wqk_t[:, m], in_=wqk[:, m])
                nc.scalar.dma_start(out=wvg_t[:], in_=wvg[:])

            def late_consts():
                nc.scalar.dma_start(out=rt_t[:], in_=rtm[:])
                nc.scalar.dma_start(out=cs_t[:], in_=cs[:])
                nc.scalar.dma_start(out=csk_t[:], in_=csk[:])
                nc.scalar.dma_start(out=tri_t[:], in_=tri[:])
                nc.scalar.dma_start(out=idn_t[:], in_=idn[:])
                nc.scalar.dma_start(out=w2_t[:], in_=w2c[:])
                nc.scalar.dma_start(out=wot_t[:], in_=wot[:])
            ones_t = consts.tile([128, 1], F32)
            nc.vector.memset(ones_t[:], 1.0)
            ones_bf = consts.tile([128, 1], BF16)
            nc.vector.memset(ones_bf[:], 1.0)
            # ones/HD in bf16 (2^-7, exact): the q-stats matmul yields mean_d
            ones_q = consts.tile([128, 1], BF16)
            nc.vector.memset(ones_q[:], 1.0 / HD)
            eps_t = consts.tile([128, 1], F32)
            nc.vector.memset(eps_t[:], EPS)
            epsh_t = consts.tile([128, 1], F32)
            nc.vector.memset(epsh_t[:], float(HD * EPS))
            inv128_t = consts.tile([128, 1], F32)
            nc.vector.memset(inv128_t[:], 1.0 / HD)
            ones_row = consts.tile([1, 128], F32R)
            nc.sync.dma_start(out=ones_row[:], in_=onesr[:])

            def qkv_ntile(nt, qk, vt):
                """project 512 tokens: q,k feature-major; v+gate token-major."""
                half = nt % 2
                xtile = load_xtile(nt)
                for m in range(4):  # q_h0, q_h1, k_h0, k_h1
                    ps = projps.tile([128, 512], F32, tag="proj")
                    for kc in range(16):
                        nc.tensor.matmul(
                            ps[:],
                            lhsT=wqk_t[:, m, kc, :],
                            rhs=xtile[:, kc, :],
                            start=(kc == 0),
                            stop=(kc == 15),
                        )
                    nc.vector.tensor_copy(
                        out=qk[:, m, half * 512 : (half + 1) * 512], in_=ps[:]
                    )
                for ti in range(4):  # v + gate logits, token-major, 128 tok each
                    ps = projps.tile([128, 512], F32, tag="proj")
                    for kc in range(16):
                        nc.tensor.matmul(
                            ps[:, 0:258],
                            lhsT=xtile[:, kc, ti * 128 : (ti + 1) * 128],
                            rhs=wvg_t[:, kc, :],
                            start=(kc == 0),
                            stop=(kc == 15),
                        )
                    nc.vector.tensor_copy(out=vt[:, half * 4 + ti, :], in_=ps[:, 0:256])
                    # gate as 1+exp(-(z+b)): shares the ACT Exp table with
                    # attention (no LoadActFuncSet thrash); the reciprocal is
                    # folded into the softmax-denominator reciprocal later
                    gsb = gsp.tile([128, 2], F32, tag="gsb")
                    t0 = nt * 512 + ti * 128
                    for h in range(2):
                        nc.scalar.activation(
                            out=gsb[:, h : h + 1],
                            in_=ps[:, 256 + h : 257 + h],
                            func=AF.Exp,
                            bias=gbn_t[:, h : h + 1],
                            scale=-1.0,
                        )
                    nc.vector.tensor_scalar_add(out=gsb[:], in0=gsb[:], scalar1=ones_t[:])
                    for h in range(2):
                        nc.sync.dma_start(
                            out=gate_scr[h : h + 1, t0 : t0 + 128],
                            in_=gsb[:, h : h + 1],
                        )

            def rope_norm(s, h, is_q, qk):
                """RoPE + RMSNorm scale for one head-tensor of one sequence.
                q: returns fin already scaled by sigma_q (broadcast multiply).
                k: returns (fin * norm_w^2, sigma_k per-partition column)."""
                m = h if is_q else 2 + h
                cst = cs_t if is_q else csk_t
                fin = ropep.tile([128, SEQ], BF16, tag="rope")
                sq = scrp.tile([128, SEQ], BF16, tag="sq")
                # rope is a per-pair rotation: it preserves sum_d q^2, so the
                # RMSNorm stats come from PRE-rope values — a chain parallel to
                # the rotation, not serial after it
                nc.gpsimd.tensor_mul(out=sq[:], in0=qk[:, m, :], in1=qk[:, m, :])
                for j in range(2):
                    js = slice(j * 512, (j + 1) * 512)
                    psr = bigps.tile([128, 512], F32, tag="big")
                    nc.tensor.matmul(
                        psr[:], lhsT=rt_t[:], rhs=qk[:, m, js], start=True, stop=True
                    )
                    nc.gpsimd.tensor_mul(
                        out=fin[:, js], in0=qk[:, m, js], in1=cst[:, 0, js]
                    )
                    tmp = scrp.tile([128, 512], F32, tag="rtmp")
                    nc.vector.tensor_mul(out=tmp[:], in0=psr[:], in1=cst[:, 1, js])
                    nc.vector.tensor_add(out=fin[:, js], in0=fin[:, js], in1=tmp[:])
                if is_q:
                    # sigma_q[t] = rsqrt(mean_d(rope_q^2) + eps), free-dim
                    # scale; processed per 512-half so the first scores tile
                    # unblocks as early as possible
                    row = rowp.tile([1, SEQ], F32, tag="qrow")
                    bc = bcp.tile([128, SEQ], F32, tag="bcq")
                    for j in range(2):
                        js = slice(j * 512, (j + 1) * 512)
                        pss = vecps.tile([1, 512], F32, tag="vec")
                        nc.tensor.matmul(
                            pss[:],
                            lhsT=ones_q[:],
                            rhs=sq[:, js],
                            start=True,
                            stop=True,
                        )
                        nc.scalar.activation(
                            out=row[:, js], in_=pss[:], func=AF.Sqrt,
                            bias=eps_t[0:1, :], scale=1.0,
                        )
                        nc.vector.reciprocal(out=row[:, js], in_=row[:, js])
                        nc.gpsimd.partition_broadcast(bc[:, js], row[:, js])
                        nc.vector.tensor_mul(
                            out=fin[:, js], in0=fin[:, js], in1=bc[:, js]
                        )
                    return fin, None
                else:
                    # sigma_k[s] = rsqrt(sum_d + HD*eps) = rstd_k/sqrt(HD),
                    # per-partition column applied inside the exp
                    col = rowp.tile([128, 8], F32, tag="kcol")
                    psc = projps.tile([128, 8], F32, tag="proj")
                    for sc in range(8):
                        nc.tensor.matmul(
                            psc[:, sc : sc + 1],
                            lhsT=sq[:, sc * 128 : (sc + 1) * 128],
                            rhs=ones_bf[:],
                            start=True,
                            stop=True,
                            skip_group_check=True,
                        )
                    nc.scalar.activation(
                        out=col[:], in_=psc[:], func=AF.Sqrt,
                        bias=epsh_t[:], scale=1.0,
                    )
                    nc.vector.reciprocal(out=col[:], in_=col[:])
                    return fin, col

            def attention(s, h, qk, vt, att, qf, kf, kcol):
                grows = []
                for tt in range(2):  # prefetch gate rows (DRAM latency off the chain)
                    grow = rowp.tile([1, 512], F32, tag="grow", name=f"grow{s}_{h}_{tt}")
                    t0 = s * SEQ + tt * 512
                    nc.sync.dma_start(
                        out=grow[:], in_=gate_scr[h : h + 1, t0 : t0 + 512]
                    )
                    grows.append(grow)
                for tt in range(2):
                    nsc = 4 * (tt + 1)
                    expt = expp.tile([128, 8, 512], BF16, tag="expt")
                    pv = bigps.tile([128, 512], F32, tag="big")
                    den = vecps.tile([1, 512], F32, tag="vec")
                    for sc in range(nsc):
                        r = sc - 4 * tt  # >= 0 on diagonal chunks
                        c0 = 128 * r if r > 0 else 0
                        sps = bigps.tile([128, 512], F32, tag="big")
                        nc.tensor.matmul(
                            sps[:, 0 : 512 - c0],
                            lhsT=kf[:, sc * 128 : (sc + 1) * 128],
                            rhs=qf[:, tt * 512 + c0 : (tt + 1) * 512],
                            start=True,
                            stop=(r < 0),
                            skip_group_check=True,
                        )
                        if r >= 0:  # diagonal chunk: accumulate the -1e30
                            # triangle on the PE itself (I.T @ tri) — keeps the
                            # scores->exp chain off the DVE
                            nc.tensor.matmul(
                                sps[:, 0:128],
                                lhsT=idn_t[:],
                                rhs=tri_t[:],
                                start=False,
                                stop=True,
                                skip_group_check=True,
                            )
                        nc.scalar.activation(
                            out=expt[:, sc, c0:512], in_=sps[:, 0 : 512 - c0],
                            func=AF.Exp, scale=kcol[:, sc : sc + 1],
                        )
                        nc.tensor.matmul(
                            den[:, c0:512],
                            lhsT=ones_bf[:],
                            rhs=expt[:, sc, c0:512],
                            start=(sc == 0),
                            stop=(sc == nsc - 1),
                            skip_group_check=True,
                        )
                        nc.tensor.matmul(
                            pv[:, c0:512],
                            lhsT=vt[:, sc, h * 128 : (h + 1) * 128],
                            rhs=expt[:, sc, c0:512],
                            start=(sc == 0),
                            stop=(sc == nsc - 1),
                            skip_group_check=True,
                        )
                    drec = rowp.tile([1, 512], F32, tag="drec")
                    nc.vector.tensor_mul(out=drec[:], in0=den[:], in1=grows[tt][:])
                    nc.vector.reciprocal(out=drec[:], in_=drec[:])
                    bcg = bcp.tile([128, 512], F32, tag="bcg")
                    nc.gpsimd.partition_broadcast(bcg[:], drec[:])
                    nc.vector.tensor_mul(
                        out=att[:, h, tt * 512 : (tt + 1) * 512], in0=pv[:], in1=bcg[:]
                    )

            def wo_proj(s, att):
                for t8 in range(8):
                    ts_ = slice(t8 * 128, (t8 + 1) * 128)
                    for ot in range(4):
                        os_ = slice(ot * 512, (ot + 1) * 512)
                        ps = bigps.tile([128, 512], F32, tag="big")
                        for h in range(2):
                            nc.tensor.matmul(
                                ps[:],
                                lhsT=att[:, h, ts_],
                                rhs=wot_t[:, h, os_],
                                start=(h == 0),
                                stop=(h == 1),
                            )
                        ob = outp.tile([128, 512], F32, tag="ob")
                        nc.scalar.copy(out=ob[:], in_=ps[:])
                        nc.sync.dma_start(
                            out=out[s * SEQ + t8 * 128 : s * SEQ + (t8 + 1) * 128, os_],
                            in_=ob[:],
                        )

            xtiles = {}

            def load_xtile(nt):
                if nt in xtiles:
                    return xtiles[nt]
                xtile = xtp.tile([128, 16, 512], BF16, tag="xtile", name=f"xt{nt}")
                # nt=0 loads in fine chunks: the very first matmuls start after
                # ~0.3MB instead of 2MB
                bounds = (0, 2, 4, 8, 12, 16) if nt == 0 else (0, 4, 8, 12, 16)
                for a, b in zip(bounds, bounds[1:]):
                    nc.sync.dma_start(
                        out=xtile[:, a:b, :],
                        in_=xt[:, a:b, nt * 512 : (nt + 1) * 512],
                    )
                xtiles[nt] = xtile
                return xtile

            for s in range(NSEQ):
                qk = qkp.tile([128, 4, SEQ], BF16, tag="qk", name=f"qk{s}")
                vt = vp.tile([128, 8, 256], BF16, tag="v", name=f"v{s}")
                att = attnp.tile([128, 2, SEQ], BF16, tag="attn")
                if s == 0:
                    # interleave the first weight/x chunks so MM(kc=0) starts
                    # after ~2 small DMAs rather than the full 2.5MB
                    nc.sync.dma_start(out=wqk_t[:, 0, 0:2], in_=wqk[:, 0, 0:2])
                    nc.scalar.dma_start(out=wqk_t[:, 0, 2:16], in_=wqk[:, 0, 2:16])
                    load_xtile(0)
                    early_consts()
                qkv_ntile(2 * s, qk, vt)
                if s == 0:
                    late_consts()
                qkv_ntile(2 * s + 1, qk, vt)
                if s + 1 < NSEQ:
                    # prefetch next seq's x tiles: the serial DMA queue is the
                    # real gate on the next projections starting promptly
                    load_xtile(2 * (s + 1))
                    load_xtile(2 * (s + 1) + 1)
                preps = []
                for h in range(2):
                    qf, _ = rope_norm(s, h, True, qk)
                    kf, kcol = rope_norm(s, h, False, qk)
                    preps.append((qf, kf, kcol))
                for h in range(2):
                    attention(s, h, qk, vt, att, *preps[h])
                wo_proj(s, att)

    if not nc.is_finalized():
        nc.finalize()
    return nc


_NC_CACHE = None


def _get_nc():
    global _NC_CACHE
    if _NC_CACHE is None:
        _NC_CACHE = build_nc()
    return _NC_CACHE


def prep_inputs(x, Wqkv, Wo, gate_w, gate_b, norm_w, cos_cache, sin_cache,
                cu_seqlens, max_seqlen, position_ids):
    x = np.asarray(x, np.float32)
    Wqkv = np.asarray(Wqkv, np.float32)
    Wo = np.asarray(Wo, np.float32)
    gate_w = np.asarray(gate_w, np.float32)
    gate_b = np.asarray(gate_b, np.float32)
    norm_w = np.asarray(norm_w, np.float32)
    cos_cache = np.asarray(cos_cache, np.float32)
    sin_cache = np.asarray(sin_cache, np.float32)
    pid = np.asarray(position_ids).astype(np.int64)
    cu = np.asarray(cu_seqlens).astype(np.int64)
    assert int(max_seqlen) == SEQ and x.shape == (N_TOK, HID)
    assert np.array_equal(cu, np.arange(NSEQ + 1, dtype=np.int64) * SEQ)
    assert np.array_equal(pid, np.tile(np.arange(SEQ, dtype=np.int64), NSEQ))

    xtf = np.ascontiguousarray(x.T).reshape(16, 128, N_TOK).transpose(1, 0, 2)
    xtf = np.ascontiguousarray(xtf).astype(BF)

    C = cos_cache[pid[:SEQ]].T  # [64, 1024]
    S = sin_cache[pid[:SEQ]].T
    csf = np.stack(
        [np.concatenate([C, C], 0), np.concatenate([S, S], 0)], axis=1
    ).astype(BF)
    w2 = (norm_w * norm_w).reshape(128, 1).astype(np.float32)
    cskf = (csf.astype(np.float32) * w2[:, None, :]).astype(BF)

    rt = np.zeros((128, 128), np.float32)
    for j in range(64):
        rt[j, 64 + j] = -1.0  # psR[64+j] = -x1[j]
        rt[64 + j, j] = 1.0  # psR[i] = x2[i]
    rt = rt.astype(BF)

    trif = np.where(
        np.arange(128)[:, None] > np.arange(128)[None, :], np.float32(-1e30), 0.0
    ).astype(BF)
    idnf = np.eye(128, dtype=np.float32).astype(BF)
    w2c = (norm_w * norm_w).reshape(128, 1).astype(np.float32)

    in_maps = []
    for c in range(NCORES):
        hs = [2 * c, 2 * c + 1]
        rows = []
        for t in range(3):  # q, k, v row blocks of Wqkv
            for h in hs:
                rows.extend(range(t * HID + h * HD, t * HID + (h + 1) * HD))
        wsel = np.concatenate([Wqkv[rows], gate_w[hs]], axis=0)  # [770, 2048]
        wall = np.ascontiguousarray(wsel.T).reshape(16, 128, 770).transpose(1, 0, 2)
        wqkf = np.ascontiguousarray(
            wall[:, :, 0:512].reshape(128, 16, 4, 128).transpose(0, 2, 1, 3)
        ).astype(BF)  # [128, 4(m), 16(kc), 128]
        wvgf = np.ascontiguousarray(wall[:, :, 512:770]).astype(BF)
        wo_sl = np.ascontiguousarray(Wo[:, c * 256 : (c + 1) * 256].T)
        wotf = wo_sl.reshape(2, 128, HID).transpose(1, 0, 2)
        wotf = np.ascontiguousarray(wotf).astype(BF)
        gbf = np.broadcast_to(-gate_b[hs][None, :], (128, 2)).astype(np.float32)
        gbf = np.ascontiguousarray(gbf)
        in_maps.append(
            {"xt": xtf, "wqk": wqkf, "wvg": wvgf, "wot": wotf, "cs": csf, "rtm": rt,
             "tri": trif, "idn": idnf, "w2c": w2c, "gbc": gbf, "csk": cskf,
             "onesr": np.ones((1, 128), np.float32)}
        )
    return in_maps


def run(inputs, trace=False):
    in_maps = prep_inputs(**inputs)
    nc = _get_nc()
    res = run_bass_kernel_spmd(nc, in_maps, core_ids=list(range(NCORES)), trace=trace)
    total = np.zeros((N_TOK, HID), np.float32)
    for c in range(NCORES):
        total += res.results[c]["out"].astype(np.float32)
    return total, res


def kernel(**inputs) -> np.ndarray:
    out, _ = run(inputs)
    return out


# revision 71
# speedup vs baseline: 1.0251x; 1.0152x over previous
"""Causal varlen self-attention (packed, equal-length) on 8 trn2 NeuronCores.

Sharding: tensor-parallel over heads — 16 heads / 8 cores = 2 heads per core.
Each core computes qkv + RoPE + RMSNorm + causal attention + sigmoid gating for
its 2 heads over all 4096 tokens, plus its partial output projection
(attn_chunk @ Wo_chunk.T).  The host sums the 8 partial outputs.

Per-core pipeline (feature-major q/k: head_dim on partitions):
  - qkv: q,k produced feature-major [d, t]; v (+ the 2 gate logits appended as
    2 extra columns of the v weight block) produced token-major [t, d].
  - RoPE via a 128x128 signed-permutation matmul + elementwise muls; RMSNorm
    partition-reductions via ones-matmuls on the PE.
  - scores computed TRANSPOSED: scoresT[s, t] = k_fin-slices.T @ q_fin so the
    k-side softmax scale folds into the exp's per-partition scale, and the
    transposed probs are exactly what the PV matmul (lhsT = token-major V)
    wants.  Softmax denominator = ones-matmul over the exp tiles.
  - causal mask: diagonal-chunk matmuls are sliced to the unmasked t-range and
    one [128,128] triangle of -1e30 is added before exp.
  - gate and 1/denominator are per-token (free-dim) scales, applied via a
    partition-broadcast SBUF->SBUF DMA then one elementwise multiply.
"""

import sys

sys.path.insert(0, "/opt/trn_rl_repo")

import numpy as np
import ml_dtypes

import concourse.bass as bass
import concourse.tile as tile
from concourse import bacc, mybir
from concourse.bass_utils import run_bass_kernel_spmd

N_TOK, HID, NH, HD = 4096, 2048, 16, 128
SEQ, NSEQ = 1024, 4
NCORES = 8
EPS = 1e-6
F32, BF16, F32R = mybir.dt.float32, mybir.dt.bfloat16, mybir.dt.float32r
BF = ml_dtypes.bfloat16
AF = mybir.ActivationFunctionType

_PATCHED = False


def _patch_tile_drain():
    """walrus in this env allows only ONE sync-wait on a TPB_CTRL instruction;
    spread the TileContext-exit drain's waits across nop instructions."""
    global _PATCHED
    if _PATCHED:
        return
    _PATCHED = True
    from concourse.tile import TileContext
    from concourse.vector_clock import ScopedClock

    def patched(self, tick_clock, wait_clock):
        nc = self.nc
        probe = nc.sync.nop(nofuse=True, hint="drain_waits_probe")
        wait_clock.add_sem_waits(probe.ins, ScopedClock({None: tick_clock.global_clock}))
        raw = list(probe.ins.sync_info.on_wait or [])
        best = {}
        for w in raw:  # keep one wait per semaphore (the largest threshold)
            k = (w.id, w.wait_mode)
            if k not in best or (w.wait_value or 0) > (best[k].wait_value or 0):
                best[k] = w
        waits = list(best.values())
        probe.ins.sync_info.on_wait = waits[:1]
        for w in waits[1:]:
            nop = nc.sync.nop(nofuse=True, hint="drain_waits")
            nop.ins.sync_info = mybir.SyncInfo(on_wait=[w], on_update=[])
        nc.sync.drain()
        nc.all_engine_barrier()
        assert self.sems is not None
        popped = nc._tile_sem_poison_stack.pop()
        assert popped is self._sem_poison
        nc.clear_and_free_semaphores(list(self.sems.allocated().values()))
        nc.all_engine_barrier()

    TileContext._drain_and_barrier = patched


def _r(ap):
    return ap.bitcast(F32R)


def build_nc():
    """One SPMD Bass program; all per-core data arrives via ExternalInputs."""
    nc = bacc.Bacc("TRN2", target_bir_lowering=False, debug=False, num_devices=NCORES)

    xt = nc.dram_tensor("xt", [128, 16, N_TOK], BF16, kind="ExternalInput")
    wqk = nc.dram_tensor("wqk", [128, 4, 16, 128], BF16, kind="ExternalInput")
    wvg = nc.dram_tensor("wvg", [128, 16, 258], BF16, kind="ExternalInput")
    wot = nc.dram_tensor("wot", [128, 2, HID], BF16, kind="ExternalInput")
    cs = nc.dram_tensor("cs", [128, 2, SEQ], BF16, kind="ExternalInput")
    csk = nc.dram_tensor("csk", [128, 2, SEQ], BF16, kind="ExternalInput")
    rtm = nc.dram_tensor("rtm", [128, 128], BF16, kind="ExternalInput")
    tri = nc.dram_tensor("tri", [128, 128], BF16, kind="ExternalInput")
    idn = nc.dram_tensor("idn", [128, 128], BF16, kind="ExternalInput")
    w2c = nc.dram_tensor("w2c", [128, 1], F32, kind="ExternalInput")
    gbc = nc.dram_tensor("gbc", [128, 2], F32, kind="ExternalInput")
    onesr = nc.dram_tensor("onesr", [1, 128], F32R, kind="ExternalInput")
    out = nc.dram_tensor("out", [N_TOK, HID], F32, kind="ExternalOutput")
    gate_scr = nc.dram_tensor("gate_scr", [2, N_TOK], F32)

    with tile.TileContext(nc) as tc:
        with (
            tc.tile_pool(name="consts", bufs=1) as consts,
            tc.tile_pool(name="xtp", bufs=3) as xtp,
            tc.tile_pool(name="qkp", bufs=2) as qkp,
            tc.tile_pool(name="vp", bufs=2) as vp,
            tc.tile_pool(name="ropep", bufs=6) as ropep,
            tc.tile_pool(name="scrp", bufs=3) as scrp,
            tc.tile_pool(name="expp", bufs=3) as expp,
            tc.tile_pool(name="attnp", bufs=2) as attnp,
            tc.tile_pool(name="outp", bufs=3) as outp,
            tc.tile_pool(name="bcp", bufs=2) as bcp,
            tc.tile_pool(name="rowp", bufs=2) as rowp,
            tc.tile_pool(name="gsp", bufs=3) as gsp,
            tc.tile_pool(name="projps", bufs=3, space="PSUM") as projps,
            tc.tile_pool(name="bigps", bufs=3, space="PSUM") as bigps,
            tc.tile_pool(name="pvps", bufs=1, space="PSUM") as pvps,
            tc.tile_pool(name="vecps", bufs=1, space="PSUM") as vecps,
        ):
            # ---- resident constants (spread across DMA queues: the first
            # qkv matmuls are gated on wc, so parallelize its load)
            # allocate const tiles now; DMA emission order is tuned so the
            # first projection chain's bytes (wqk m=0 + x tile 0) land first
            wqk_t = consts.tile([128, 4, 16, 128], BF16)
            wvg_t = consts.tile([128, 16, 258], BF16)
            wot_t = consts.tile([128, 2, HID], BF16)
            cs_t = consts.tile([128, 2, SEQ], BF16)
            csk_t = consts.tile([128, 2, SEQ], BF16)
            rt_t = consts.tile([128, 128], BF16)
            tri_t = consts.tile([128, 128], BF16)
            idn_t = consts.tile([128, 128], BF16)
            w2_t = consts.tile([128, 1], F32)
            gbn_t = consts.tile([128, 2], F32)
            nc.sync.dma_start(out=gbn_t[:], in_=gbc[:])

            def early_consts():
                # emitted after the first x-tile DMA: wqk m=0 + x tile 0 land
                # first on the serial DMA queue, then the rest of the weights.
                # These MUST be emitted before qkv_ntile(0)'s matmuls — Tile
                # tracks RAW deps in emission order.
                for m in range(1, 4):
                    eng = nc.sync if m % 2 == 0 else nc.scalar
                    eng.dma_start(out=wqk_t[:, m], in_=wqk[:, m])
                nc.scalar.dma_start(out=wvg_t[:], in_=wvg[:])

            def late_consts():
                nc.scalar.dma_start(out=rt_t[:], in_=rtm[:])
                nc.scalar.dma_start(out=cs_t[:], in_=cs[:])
                nc.scalar.dma_start(out=csk_t[:], in_=csk[:])
                nc.scalar.dma_start(out=tri_t[:], in_=tri[:])
                nc.scalar.dma_start(out=idn_t[:], in_=idn[:])
                nc.scalar.dma_start(out=w2_t[:], in_=w2c[:])
                nc.scalar.dma_start(out=wot_t[:], in_=wot[:])
            ones_t = consts.tile([128, 1], F32)
            nc.vector.memset(ones_t[:], 1.0)
            ones_bf = consts.tile([128, 1], BF16)
            nc.vector.memset(ones_bf[:], 1.0)
            # ones/HD in bf16 (2^-7, exact): the q-stats matmul yields mean_d
            ones_q = consts.tile([128, 1], BF16)
            nc.vector.memset(ones_q[:], 1.0 / HD)
            eps_t = consts.tile([128, 1], F32)
            nc.vector.memset(eps_t[:], EPS)
            epsh_t = consts.tile([128, 1], F32)
            nc.vector.memset(epsh_t[:], float(HD * EPS))
            inv128_t = consts.tile([128, 1], F32)
            nc.vector.memset(inv128_t[:], 1.0 / HD)
            ones_row = consts.tile([1, 128], F32R)
            nc.sync.dma_start(out=ones_row[:], in_=onesr[:])

            def qkv_ntile(nt, qk, vt):
                """project 512 tokens: q,k feature-major; v+gate token-major."""
                half = nt % 2
                xtile = load_xtile(nt)
                for m in range(4):  # q_h0, q_h1, k_h0, k_h1
                    ps = projps.tile([128, 512], F32, tag="proj")
                    for kc in range(16):
                        nc.tensor.matmul(
                            ps[:],
                            lhsT=wqk_t[:, m, kc, :],
                            rhs=xtile[:, kc, :],
                            start=(kc == 0),
                            stop=(kc == 15),
                        )
                    nc.vector.tensor_copy(
                        out=qk[:, m, half * 512 : (half + 1) * 512], in_=ps[:]
                    )
                for ti in range(4):  # v + gate logits, token-major, 128 tok each
                    ps = projps.tile([128, 512], F32, tag="proj")
                    for kc in range(16):
                        nc.tensor.matmul(
                            ps[:, 0:258],
                            lhsT=xtile[:, kc, ti * 128 : (ti + 1) * 128],
                            rhs=wvg_t[:, kc, :],
                            start=(kc == 0),
                            stop=(kc == 15),
                        )
                    nc.vector.tensor_copy(out=vt[:, half * 4 + ti, :], in_=ps[:, 0:256])
                    # gate as 1+exp(-(z+b)): shares the ACT Exp table with
                    # attention (no LoadActFuncSet thrash); the reciprocal is
                    # folded into the softmax-denominator reciprocal later
                    gsb = gsp.tile([128, 2], F32, tag="gsb")
                    t0 = nt * 512 + ti * 128
                    for h in range(2):
                        nc.scalar.activation(
                            out=gsb[:, h : h + 1],
                            in_=ps[:, 256 + h : 257 + h],
                            func=AF.Exp,
                            bias=gbn_t[:, h : h + 1],
                            scale=-1.0,
                        )
                    nc.vector.tensor_scalar_add(out=gsb[:], in0=gsb[:], scalar1=ones_t[:])
                    for h in range(2):
                        nc.sync.dma_start(
                            out=gate_scr[h : h + 1, t0 : t0 + 128],
                            in_=gsb[:, h : h + 1],
                        )

            def rope_norm(s, h, is_q, qk):
                """RoPE + RMSNorm scale for one head-tensor of one sequence.
                q: returns fin already scaled by sigma_q (broadcast multiply).
                k: returns (fin * norm_w^2, sigma_k per-partition column)."""
                m = h if is_q else 2 + h
                cst = cs_t if is_q else csk_t
                fin = ropep.tile([128, SEQ], BF16, tag="rope")
                sq = scrp.tile([128, SEQ], BF16, tag="sq")
                # rope is a per-pair rotation: it preserves sum_d q^2, so the
                # RMSNorm stats come from PRE-rope values — a chain parallel to
                # the rotation, not serial after it
                nc.gpsimd.tensor_mul(out=sq[:], in0=qk[:, m, :], in1=qk[:, m, :])
                for j in range(2):
                    js = slice(j * 512, (j + 1) * 512)
                    psr = bigps.tile([128, 512], F32, tag="big")
                    nc.tensor.matmul(
                        psr[:], lhsT=rt_t[:], rhs=qk[:, m, js], start=True, stop=True
                    )
                    nc.gpsimd.tensor_mul(
                        out=fin[:, js], in0=qk[:, m, js], in1=cst[:, 0, js]
                    )
                    tmp = scrp.tile([128, 512], F32, tag="rtmp")
                    nc.vector.tensor_mul(out=tmp[:], in0=psr[:], in1=cst[:, 1, js])
                    nc.vector.tensor_add(out=fin[:, js], in0=fin[:, js], in1=tmp[:])
                if is_q:
                    # sigma_q[t] = rsqrt(mean_d(rope_q^2) + eps), free-dim
                    # scale; processed per 512-half so the first scores tile
                    # unblocks as early as possible
                    row = rowp.tile([1, SEQ], F32, tag="qrow")
                    bc = bcp.tile([128, SEQ], F32, tag="bcq")
                    for j in range(2):
                        js = slice(j * 512, (j + 1) * 512)
                        pss = vecps.tile([1, 512], F32, tag="vec")
                        nc.tensor.matmul(
                            pss[:],
                            lhsT=ones_q[:],
                            rhs=sq[:, js],
                            start=True,
                            stop=True,
                        )
                        nc.scalar.activation(
                            out=row[:, js], in_=pss[:], func=AF.Sqrt,
                            bias=eps_t[0:1, :], scale=1.0,
                        )
                        nc.vector.reciprocal(out=row[:, js], in_=row[:, js])
                        nc.gpsimd.partition_broadcast(bc[:, js], row[:, js])
                        nc.vector.tensor_mul(
                            out=fin[:, js], in0=fin[:, js], in1=bc[:, js]
                        )
                    return fin, None
                else:
                    # sigma_k[s] = rsqrt(sum_d + HD*eps) = rstd_k/sqrt(HD),
                    # per-partition column applied inside the exp
                    col = rowp.tile([128, 8], F32, tag="kcol")
                    psc = projps.tile([128, 8], F32, tag="proj")
                    for sc in range(8):
                        nc.tensor.matmul(
                            psc[:, sc : sc + 1],
                            lhsT=sq[:, sc * 128 : (sc + 1) * 128],
                            rhs=ones_bf[:],
                            start=True,
                            stop=True,
                            skip_group_check=True,
                        )
                    nc.scalar.activation(
                        out=col[:], in_=psc[:], func=AF.Sqrt,
                        bias=epsh_t[:], scale=1.0,
                    )
                    nc.vector.reciprocal(out=col[:], in_=col[:])
                    return fin, col

            def attention(s, h, qk, vt, att, qf, kf, kcol):
                grows = []
                for tt in range(2):  # prefetch gate rows (DRAM latency off the chain)
                    grow = rowp.tile([1, 512], F32, tag="grow", name=f"grow{s}_{h}_{tt}")
                    t0 = s * SEQ + tt * 512
                    nc.sync.dma_start(
                        out=grow[:], in_=gate_scr[h : h + 1, t0 : t0 + 512]
                    )
                    grows.append(grow)
                for tt in range(2):
                    nsc = 4 * (tt + 1)
                    expt = expp.tile([128, 8, 512], BF16, tag="expt")
                    pv = pvps.tile([128, 512], F32, tag="pv")
                    den = vecps.tile([1, 512], F32, tag="vec")
                    for sc in range(nsc):
                        r = sc - 4 * tt  # >= 0 on diagonal chunks
                        c0 = 128 * r if r > 0 else 0
                        sps = bigps.tile([128, 512], F32, tag="big")
                        nc.tensor.matmul(
                            sps[:, 0 : 512 - c0],
                            lhsT=kf[:, sc * 128 : (sc + 1) * 128],
                            rhs=qf[:, tt * 512 + c0 : (tt + 1) * 512],
                            start=True,
                            stop=(r < 0),
                            skip_group_check=True,
                        )
                        if r >= 0:  # diagonal chunk: accumulate the -1e30
                            # triangle on the PE itself (I.T @ tri) — keeps the
                            # scores->exp chain off the DVE
                            nc.tensor.matmul(
                                sps[:, 0:128],
                                lhsT=idn_t[:],
                                rhs=tri_t[:],
                                start=False,
                                stop=True,
                                skip_group_check=True,
                            )
                        nc.scalar.activation(
                            out=expt[:, sc, c0:512], in_=sps[:, 0 : 512 - c0],
                            func=AF.Exp, scale=kcol[:, sc : sc + 1],
                        )
                        nc.tensor.matmul(
                            den[:, c0:512],
                            lhsT=ones_bf[:],
                            rhs=expt[:, sc, c0:512],
                            start=(sc == 0),
                            stop=(sc == nsc - 1),
                            skip_group_check=True,
                        )
                        nc.tensor.matmul(
                            pv[:, c0:512],
                            lhsT=vt[:, sc, h * 128 : (h + 1) * 128],
                            rhs=expt[:, sc, c0:512],
                            start=(sc == 0),
                            stop=(sc == nsc - 1),
                            skip_group_check=True,
                        )
                    drec = rowp.tile([1, 512], F32, tag="drec")
                    nc.vector.tensor_mul(out=drec[:], in0=den[:], in1=grows[tt][:])
                    nc.vector.reciprocal(out=drec[:], in_=drec[:])
                    bcg = bcp.tile([128, 512], F32, tag="bcg")
                    nc.gpsimd.partition_broadcast(bcg[:], drec[:])
                    nc.vector.tensor_mul(
                        out=att[:, h, tt * 512 : (tt + 1) * 512], in0=pv[:], in1=bcg[:]
                    )

            def wo_proj(s, att):
                for t8 in range(8):
                    ts_ = slice(t8 * 128, (t8 + 1) * 128)
                    for ot in range(4):
                        os_ = slice(ot * 512, (ot + 1) * 512)
                        ps = bigps.tile([128, 512], F32, tag="big")
                        for h in range(2):
                            nc.tensor.matmul(
                                ps[:],
                                lhsT=att[:, h, ts_],
                                rhs=wot_t[:, h, os_],
                                start=(h == 0),
                                stop=(h == 1),
                            )
                        ob = outp.tile([128, 512], F32, tag="ob")
                        nc.scalar.copy(out=ob[:], in_=ps[:])
                        nc.sync.dma_start(
                            out=out[s * SEQ + t8 * 128 : s * SEQ + (t8 + 1) * 128, os_],
                            in_=ob[:],
                        )

            xtiles = {}

            def load_xtile(nt):
                if nt in xtiles:
                    return xtiles[nt]
                xtile = xtp.tile([128, 16, 512], BF16, tag="xtile", name=f"xt{nt}")
                # nt=0 loads in fine chunks: the very first matmuls start after
                # ~0.3MB instead of 2MB
                bounds = (0, 2, 4, 8, 12, 16) if nt == 0 else (0, 4, 8, 12, 16)
                for a, b in zip(bounds, bounds[1:]):
                    nc.sync.dma_start(
                        out=xtile[:, a:b, :],
                        in_=xt[:, a:b, nt * 512 : (nt + 1) * 512],
                    )
                xtiles[nt] = xtile
                return xtile

            for s in range(NSEQ):
                qk = qkp.tile([128, 4, SEQ], BF16, tag="qk", name=f"qk{s}")
                vt = vp.tile([128, 8, 256], BF16, tag="v", name=f"v{s}")
                att = attnp.tile([128, 2, SEQ], BF16, tag="attn")
                if s == 0:
                    # interleave the first weight/x chunks so MM(kc=0) starts
                    # after ~2 small DMAs rather than the full 2.5MB
                    nc.sync.dma_start(out=wqk_t[:, 0, 0:2], in_=wqk[:, 0, 0:2])
                    nc.scalar.dma_start(out=wqk_t[:, 0, 2:16], in_=wqk[:, 0, 2:16])
                    load_xtile(0)
                    early_consts()
                qkv_ntile(2 * s, qk, vt)
                if s == 0:
                    late_consts()
                qkv_ntile(2 * s + 1, qk, vt)
                if s + 1 < NSEQ:
                    # prefetch next seq's x tiles: the serial DMA queue is the
                    # real gate on the next projections starting promptly
                    load_xtile(2 * (s + 1))
                    load_xtile(2 * (s + 1) + 1)
                preps = []
                for h in range(2):
                    qf, _ = rope_norm(s, h, True, qk)
                    kf, kcol = rope_norm(s, h, False, qk)
                    preps.append((qf, kf, kcol))
                for h in range(2):
                    attention(s, h, qk, vt, att, *preps[h])
                wo_proj(s, att)

    if not nc.is_finalized():
        nc.finalize()
    return nc


_NC_CACHE = None


def _get_nc():
    global _NC_CACHE
    if _NC_CACHE is None:
        _NC_CACHE = build_nc()
    return _NC_CACHE


def prep_inputs(x, Wqkv, Wo, gate_w, gate_b, norm_w, cos_cache, sin_cache,
                cu_seqlens, max_seqlen, position_ids):
    x = np.asarray(x, np.float32)
    Wqkv = np.asarray(Wqkv, np.float32)
    Wo = np.asarray(Wo, np.float32)
    gate_w = np.asarray(gate_w, np.float32)
    gate_b = np.asarray(gate_b, np.float32)
    norm_w = np.asarray(norm_w, np.float32)
    cos_cache = np.asarray(cos_cache, np.float32)
    sin_cache = np.asarray(sin_cache, np.float32)
    pid = np.asarray(position_ids).astype(np.int64)
    cu = np.asarray(cu_seqlens).astype(np.int64)
    assert int(max_seqlen) == SEQ and x.shape == (N_TOK, HID)
    assert np.array_equal(cu, np.arange(NSEQ + 1, dtype=np.int64) * SEQ)
    assert np.array_equal(pid, np.tile(np.arange(SEQ, dtype=np.int64), NSEQ))

    xtf = np.ascontiguousarray(x.T).reshape(16, 128, N_TOK).transpose(1, 0, 2)
    xtf = np.ascontiguousarray(xtf).astype(BF)

    C = cos_cache[pid[:SEQ]].T  # [64, 1024]
    S = sin_cache[pid[:SEQ]].T
    csf = np.stack(
        [np.concatenate([C, C], 0), np.concatenate([S, S], 0)], axis=1
    ).astype(BF)
    w2 = (norm_w * norm_w).reshape(128, 1).astype(np.float32)
    cskf = (csf.astype(np.float32) * w2[:, None, :]).astype(BF)

    rt = np.zeros((128, 128), np.float32)
    for j in range(64):
        rt[j, 64 + j] = -1.0  # psR[64+j] = -x1[j]
        rt[64 + j, j] = 1.0  # psR[i] = x2[i]
    rt = rt.astype(BF)

    trif = np.where(
        np.arange(128)[:, None] > np.arange(128)[None, :], np.float32(-1e30), 0.0
    ).astype(BF)
    idnf = np.eye(128, dtype=np.float32).astype(BF)
    w2c = (norm_w * norm_w).reshape(128, 1).astype(np.float32)

    in_maps = []
    for c in range(NCORES):
        hs = [2 * c, 2 * c + 1]
        rows = []
        for t in range(3):  # q, k, v row blocks of Wqkv
            for h in hs:
                rows.extend(range(t * HID + h * HD, t * HID + (h + 1) * HD))
        wsel = np.concatenate([Wqkv[rows], gate_w[hs]], axis=0)  # [770, 2048]
        wall = np.ascontiguousarray(wsel.T).reshape(16, 128, 770).transpose(1, 0, 2)
        wqkf = np.ascontiguousarray(
            wall[:, :, 0:512].reshape(128, 16, 4, 128).transpose(0, 2, 1, 3)
        ).astype(BF)  # [128, 4(m), 16(kc), 128]
        wvgf = np.ascontiguousarray(wall[:, :, 512:770]).astype(BF)
        wo_sl = np.ascontiguousarray(Wo[:, c * 256 : (c + 1) * 256].T)
        wotf = wo_sl.reshape(2, 128, HID).transpose(1, 0, 2)
        wotf = np.ascontiguousarray(wotf).astype(BF)
        gbf = np.broadcast_to(-gate_b[hs][None, :], (128, 2)).astype(np.float32)
        gbf = np.ascontiguousarray(gbf)
        in_maps.append(
            {"xt": xtf, "wqk": wqkf, "wvg": wvgf, "wot": wotf, "cs": csf, "rtm": rt,
             "tri": trif, "idn": idnf, "w2c": w2c, "gbc": gbf, "csk": cskf,
             "onesr": np.ones((1, 128), np.float32)}
        )
    return in_maps


def run(inputs, trace=False):
    in_maps = prep_inputs(**inputs)
    nc = _get_nc()
    res = run_bass_kernel_spmd(nc, in_maps, core_ids=list(range(NCORES)), trace=trace)
    total = np.zeros((N_TOK, HID), np.float32)
    for c in range(NCORES):
        total += res.results[c]["out"].astype(np.float32)
    return total, res


def kernel(**inputs) -> np.ndarray:
    out, _ = run(inputs)
    return out
